# revision 29
# baseline (speedup 1.0000x reference)
"""Trainium2 Bass kernel for nn_ActionScoringModel (LRU + max-pool + tanh MLP).

Strategy: data-parallel over batch (64 = 8 cores x 8 batches). No collectives.
V3 pipeline (per core):
  - Host de-interleaves sequence positions per 128-chunk:
    obsT chunk cols = [odd s=1,3..2047 | even s=0,2..2046]. All device-side
    decimation views become contiguous -> DVE 2x_1p mode.
  - statA: u = A @ obsT per 512-block (i-outer, k-inner), uAc bf16 copy on
    scalar with pad cols 0/1025 zeroed.
  - rotate-in: t1 = uAc (.) cosDI (DVE 2x), t2 = uAc (.) sinDI (gpsimd);
    rho for the scan-pair fold is pre-scaled into the odd-half table cols.
  - decimation x2 as contiguous TT-adds: q1 = t1_odd + t1_even,
    q2 = t2_odd + t2_even (DVE 2x). PE mix: wE = I@q1 + P@q2 (4 MMs/batch,
    half the V2 count).
  - DVE scan length 1024 with rho^2; p1 = g (.) cosE (DVE), p2 = g (.) sinE
    (gpsimd).
  - y per 512-block: even plane = cm1@p1+cm2@p2+D@obs_even; odd plane =
    cm1l@p1+cm2l@p2+cm1@uAc_odd+D@obs_odd; tensor MAX reduces; per-batch
    pair-max via permP matmul.
  - per-batch tanh MLP head right after each reduce (no half-batch barrier).
  - startup: batch-0 obsT split in 6 pieces across sync/scalar/gpsimd DMA
    rings; tables ordered by first-use time; xa (action-side MLP input)
    emitted at i==2.
"""

import sys
import numpy as np
from contextlib import ExitStack

for _p in ("/opt/trn_rl_repo",):
    if _p not in sys.path:
        sys.path.insert(0, _p)

import ml_dtypes
import concourse.bass as bass
import concourse.tile as tile
from concourse import bacc, mybir
from concourse.bass_utils import run_bass_kernel_spmd

BF16 = mybir.dt.bfloat16
F16 = mybir.dt.float16
F32 = mybir.dt.float32

B_, S_, A_, D_IN, H_, D_OUT, D_MLP = 64, 2048, 128, 384, 64, 64, 64
NCORES = 8
NB = B_ // NCORES          # 8 batches per core
NDC = D_IN // 128          # 3 d-chunks
SH = S_ // 2               # 1024 decimated length
W_ = S_ + 2                # uAc/t1/t2 width: pad | odd 1024 | pad | even 1024
NAB = NB * A_              # 1024 action rows per core


def _build_nc():
    nc = bacc.Bacc("TRN2", target_bir_lowering=False, debug=False,
                   num_devices=1)

    # ---- DRAM I/O ----
    obsT_d = nc.dram_tensor("obsT", [NB, NDC, 128, S_], BF16,
                            kind="ExternalInput").ap()
    actT_d = nc.dram_tensor("actT", [NDC, 128, NAB], BF16,
                            kind="ExternalInput").ap()
    tabs_d = nc.dram_tensor("tabs", [128, 2 * W_], BF16,
                            kind="ExternalInput").ap()
    rho2f_d = nc.dram_tensor("rho2f", [128, 512], F32,
                             kind="ExternalInput").ap()
    stat_d = nc.dram_tensor("stat", [128, 1344], BF16,
                            kind="ExternalInput").ap()
    w2_d = nc.dram_tensor("w2", [64, 32], BF16, kind="ExternalInput").ap()
    w3_d = nc.dram_tensor("w3", [32, 1], BF16, kind="ExternalInput").ap()
    b1_d = nc.dram_tensor("b1", [64, 1], F32, kind="ExternalInput").ap()
    b2_d = nc.dram_tensor("b2", [32, 1], F32, kind="ExternalInput").ap()
    b3_d = nc.dram_tensor("b3", [1, 1], F32, kind="ExternalInput").ap()
    out_d = nc.dram_tensor("out", [1, NAB], F32, kind="ExternalOutput").ap()

    MULT = mybir.AluOpType.mult
    ADD = mybir.AluOpType.add
    MAX = mybir.AluOpType.max
    TANH = mybir.ActivationFunctionType.Tanh
    X = mybir.AxisListType.X

    with tile.TileContext(nc) as tc, ExitStack() as ctx:
        const = ctx.enter_context(tc.tile_pool(name="const", bufs=1))
        obsT_pool = ctx.enter_context(tc.tile_pool(name="obsT", bufs=5))
        uac_pool = ctx.enter_context(tc.tile_pool(name="uac", bufs=3))
        rot_pool = ctx.enter_context(tc.tile_pool(name="rot", bufs=2))
        q_pool = ctx.enter_context(tc.tile_pool(name="qp", bufs=2))
        g_pool = ctx.enter_context(tc.tile_pool(name="gp", bufs=2))
        p_pool = ctx.enter_context(tc.tile_pool(name="pp", bufs=3))
        small = ctx.enter_context(tc.tile_pool(name="small", bufs=1))
        pUA = ctx.enter_context(tc.tile_pool(name="pUA", bufs=1, space="PSUM"))
        pWE = ctx.enter_context(tc.tile_pool(name="pWE", bufs=1, space="PSUM"))
        pY = ctx.enter_context(tc.tile_pool(name="pY", bufs=1, space="PSUM"))

        # ---- const tiles ----
        stat = const.tile([128, 1344], BF16, tag="stat", name="stat")
        tabs = const.tile([128, 2 * W_], BF16, tag="tabs", name="tabs")
        rho2f = const.tile([128, 512], F32, tag="rho2f", name="rho2f")
        actT = [const.tile([128, NAB], BF16, tag=f"actT{k}", name=f"actT{k}")
                for k in range(NDC)]
        w2 = const.tile([64, 32], BF16, tag="w2", name="w2")
        w3 = const.tile([32, 1], BF16, tag="w3", name="w3")
        b1 = const.tile([64, 1], F32, tag="b1", name="b1")
        b2 = const.tile([32, 1], F32, tag="b2", name="b2")
        b3 = const.tile([1, 1], F32, tag="b3", name="b3")

        statA = [stat[:, k * 128:(k + 1) * 128] for k in range(NDC)]
        permP = stat[:, 384:512]
        ident = stat[:, 512:640]
        statD = [stat[:, 640 + k * 64:640 + (k + 1) * 64] for k in range(NDC)]
        cm1 = stat[:, 832:896]
        cm2 = stat[:, 896:960]
        cm1l = stat[:, 960:1024]
        cm2l = stat[:, 1024:1088]
        w1lat = stat[:, 1088:1152]
        w1act = [stat[:, 1152 + k * 64:1152 + (k + 1) * 64]
                 for k in range(NDC)]

        cosDI = tabs[:, 0:W_]
        sinDI = tabs[:, W_:2 * W_]
        # even-position rotate-out tables == the even halves of the DI
        # tables (sinE row-sign pattern is folded into cm2/cm2l on host)
        cosE = tabs[:, 1026:W_]
        sinE = tabs[:, W_ + 1026:2 * W_]

        # persistent result tiles
        lat128 = small.tile([128, NB], F32, tag="lat128", name="lat128")
        lat128b = small.tile([128, NB], BF16, tag="lat128b", name="lat128b")
        latf = small.tile([64, NB], F32, tag="latf", name="latf")
        latfb = small.tile([64, NB], BF16, tag="latfb", name="latfb")
        latW = small.tile([64, NB], F32, tag="latW", name="latW")
        xa = small.tile([64, NAB], F32, tag="xa", name="xa")
        x1 = small.tile([64, NAB], BF16, tag="x1", name="x1")
        x2 = small.tile([32, NAB], BF16, tag="x2", name="x2")
        x3 = small.tile([1, NAB], F32, tag="x3", name="x3")
        ymax = small.tile([128, 2 * NB], F32, tag="ymax", name="ymax")
        scr = small.tile([1, 4], F32, tag="scr", name="scr")

        # ---- startup DMAs ----
        # A dma_start BLOCKS its issuing engine until earlier transfers on
        # the same ring complete, so each engine only issues DMAs it is
        # about to need itself (self-pacing); bulk obsT traffic lives on
        # the otherwise-idle sync ring.
        # sync:   b0 k0 pieces -> b1 k0/k1 -> stat rest -> actT -> smalls
        #         -> b2..b7 k0/k1 (in loop)
        # scalar: statA -> b0 k1 pieces -> perm/ident  (then uAc copies)
        # gpsimd: b0 k2 pieces -> b1 k2 -> sinE        (then q2/p2)
        # vector: cosDI/sinDI halves -> rho2f -> cosE  (then t1/t2/q1/scans)
        obsT_t = {}

        def alloc_obs(b):
            obsT_t[b] = obsT_pool.tile([128, NDC * S_], BF16, tag="obsT",
                                       name=f"obsT{b}")

        def load(b):
            alloc_obs(b)
            for k in range(2):
                nc.sync.dma_start(out=obsT_t[b][:, k * S_:(k + 1) * S_],
                                  in_=obsT_d[b, k])
            nc.gpsimd.dma_start(out=obsT_t[b][:, 2 * S_:3 * S_],
                                in_=obsT_d[b, 2])

        ring = [nc.sync, nc.scalar, nc.gpsimd]
        nc.scalar.dma_start(out=stat[:, 0:384], in_=stat_d[:, 0:384])
        alloc_obs(0)
        for half in range(2):
            hs = slice(half * 1024, (half + 1) * 1024)
            for k in range(NDC):
                ring[k].dma_start(
                    out=obsT_t[0][:, k * S_ + half * 1024:
                                  k * S_ + (half + 1) * 1024],
                    in_=obsT_d[0, k][:, hs])
        nc.scalar.dma_start(out=stat[:, 384:640], in_=stat_d[:, 384:640])
        nc.scalar.dma_start(out=rho2f[:], in_=rho2f_d)
        # gps ring: DI tables interleaved with b1 k2 (self-paced before its
        # q2/p2 work, which isn't needed until ~18us)
        nc.gpsimd.dma_start(out=tabs[:, 0:1026], in_=tabs_d[:, 0:1026])
        nc.gpsimd.dma_start(out=tabs[:, W_:W_ + 1026],
                            in_=tabs_d[:, W_:W_ + 1026])
        alloc_obs(1)
        for k in range(2):
            nc.sync.dma_start(out=obsT_t[1][:, k * S_:(k + 1) * S_],
                              in_=obsT_d[1, k])
        nc.gpsimd.dma_start(out=obsT_t[1][:, 2 * S_:3 * S_],
                            in_=obsT_d[1, 2])
        nc.gpsimd.dma_start(out=tabs[:, 1026:W_], in_=tabs_d[:, 1026:W_])
        nc.gpsimd.dma_start(out=tabs[:, W_ + 1026:2 * W_],
                            in_=tabs_d[:, W_ + 1026:2 * W_])
        # remaining consts on sync, behind b1
        nc.sync.dma_start(out=stat[:, 640:1344], in_=stat_d[:, 640:1344])
        for k in range(NDC):
            nc.sync.dma_start(out=actT[k][:], in_=actT_d[k])
        nc.sync.dma_start(out=w2[:], in_=w2_d)
        nc.sync.dma_start(out=w3[:], in_=w3_d)
        nc.sync.dma_start(out=b1[:], in_=b1_d)
        nc.sync.dma_start(out=b2[:], in_=b2_d)
        nc.sync.dma_start(out=b3[:], in_=b3_d)

        st = {}

        # uAc segment for uA block ib: pad|odd odd|pad|even even
        USEG = [slice(1, 513), slice(513, 1025), slice(1026, 1538),
                slice(1538, 2050)]

        def stage_statA_rot(b):
            s = st.setdefault(b, {})
            obs = obsT_t[b]
            uAc = uac_pool.tile([128, W_], BF16, tag="uAc", name="uAc")
            nc.gpsimd.memset(uAc[:, 0:1], 0.0)
            nc.gpsimd.memset(uAc[:, 1025:1026], 0.0)
            t1 = rot_pool.tile([128, W_], BF16, tag="t1", name="t1")
            t2 = rot_pool.tile([128, W_], BF16, tag="t2", name="t2")
            uA = [None] * 4
            for ib in range(4):
                uA[ib] = pUA.tile([128, 512], F32, tag=f"uA{ib}",
                                  name=f"uA{ib}")
                for k in range(NDC):
                    nc.tensor.matmul(
                        out=uA[ib][:], lhsT=statA[k],
                        rhs=obs[:, k * S_ + ib * 512:k * S_ + (ib + 1) * 512],
                        start=(k == 0), stop=(k == NDC - 1))
                nc.scalar.copy(out=uAc[:, USEG[ib]], in_=uA[ib][:])
                if ib == 1:
                    nc.vector.tensor_tensor(out=t1[:, 0:1026],
                                            in0=uAc[:, 0:1026],
                                            in1=cosDI[:, 0:1026], op=MULT)
                    nc.vector.tensor_tensor(out=t2[:, 0:1026],
                                            in0=uAc[:, 0:1026],
                                            in1=sinDI[:, 0:1026], op=MULT)
                elif ib == 3:
                    nc.vector.tensor_tensor(out=t1[:, 1026:W_],
                                            in0=uAc[:, 1026:W_],
                                            in1=cosDI[:, 1026:W_], op=MULT)
                    nc.vector.tensor_tensor(out=t2[:, 1026:W_],
                                            in0=uAc[:, 1026:W_],
                                            in1=sinDI[:, 1026:W_], op=MULT)
            q1 = q_pool.tile([128, SH], BF16, tag="q1", name="q1")
            q2 = q_pool.tile([128, SH], BF16, tag="q2", name="q2")
            nc.vector.tensor_tensor(out=q1[:], in0=t1[:, 0:1024],
                                    in1=t1[:, 1026:W_], op=ADD)
            # q2 on gpsimd, guarded to start after q1 so the two engines
            # never contend for SBUF ports (4R+2W concurrent = ~3x slowdown)
            nc.gpsimd.tensor_copy(out=scr[:, 0:1], in_=q1[0:1, 1023:1024])
            nc.gpsimd.tensor_tensor(out=q2[:], in0=t2[:, 0:1024],
                                    in1=t2[:, 1026:W_], op=ADD)
            s["uAc"], s["q1"], s["q2"] = uAc, q1, q2

        def stage_mix_scan(b):
            s = st[b]
            wE = [pWE.tile([128, 512], F32, tag=f"wE{j}", name=f"wE{j}")
                  for j in range(2)]
            for j in range(2):
                nc.tensor.matmul(out=wE[j][:], lhsT=ident,
                                 rhs=s["q1"][:, j * 512:(j + 1) * 512],
                                 start=True, stop=False)
            for j in range(2):
                nc.tensor.matmul(out=wE[j][:], lhsT=permP,
                                 rhs=s["q2"][:, j * 512:(j + 1) * 512],
                                 start=False, stop=True)
            g = g_pool.tile([128, SH], BF16, tag="g", name="g")
            nc.vector.tensor_tensor_scan(out=g[:, 0:512], data0=rho2f[:],
                                         data1=wE[0][:], initial=0.0,
                                         op0=MULT, op1=ADD)
            nc.vector.tensor_tensor_scan(out=g[:, 512:SH], data0=rho2f[:],
                                         data1=wE[1][:],
                                         initial=g[:, 511:512],
                                         op0=MULT, op1=ADD)
            s["g"] = g

        def stage_p(b):
            s = st[b]
            g = s["g"]
            p1 = p_pool.tile([128, SH], BF16, tag="p1", name="p1")
            p2 = p_pool.tile([128, SH], BF16, tag="p2", name="p2")
            nc.vector.tensor_tensor(out=p1[:], in0=g[:], in1=cosE[:], op=MULT)
            # p2 on gpsimd, guarded after p1 (same port-conflict avoidance)
            nc.gpsimd.tensor_copy(out=scr[:, 1:2], in_=p1[0:1, 1023:1024])
            nc.gpsimd.tensor_tensor(out=p2[:], in0=g[:], in1=sinE[:], op=MULT)
            s["p1"], s["p2"] = p1, p2

        def emit_xa():
            for half in range(2):
                hl = slice(half * 512, (half + 1) * 512)
                pxa = pWE.tile([128, 512], F32, tag=f"wE{half}", name="pxa")
                for k in range(NDC):
                    nc.tensor.matmul(out=pxa[:64, :], lhsT=w1act[k],
                                     rhs=actT[k][:, hl],
                                     start=(k == 0), stop=(k == NDC - 1))
                nc.scalar.copy(out=xa[:, hl], in_=pxa[:64, :])

        def stage_y_red(b):
            s = st[b]
            obs = obsT_t[b]
            uAc, p1, p2 = s["uAc"], s["p1"], s["p2"]
            for blk in range(2):
                sl = slice(blk * 512, (blk + 1) * 512)
                py = pY.tile([128, 512], F32, tag=f"pY{blk}",
                             name=f"pY{blk}")
                # even plane -> partitions 0:64
                nc.tensor.matmul(out=py[0:64, :], lhsT=cm1, rhs=p1[:, sl],
                                 start=True, stop=False)
                nc.tensor.matmul(out=py[0:64, :], lhsT=cm2, rhs=p2[:, sl],
                                 start=False, stop=False)
                for k in range(NDC):
                    nc.tensor.matmul(
                        out=py[0:64, :], lhsT=statD[k],
                        rhs=obs[:, k * S_ + 1024 + blk * 512:
                                k * S_ + 1024 + (blk + 1) * 512],
                        start=False, stop=(k == NDC - 1))
                # odd plane -> partitions 64:128
                nc.tensor.matmul(out=py[64:128, :], lhsT=cm1l, rhs=p1[:, sl],
                                 start=True, stop=False)
                nc.tensor.matmul(out=py[64:128, :], lhsT=cm2l, rhs=p2[:, sl],
                                 start=False, stop=False)
                nc.tensor.matmul(out=py[64:128, :], lhsT=cm1,
                                 rhs=uAc[:, 1 + blk * 512:1 + (blk + 1) * 512],
                                 start=False, stop=False)
                for k in range(NDC):
                    nc.tensor.matmul(
                        out=py[64:128, :], lhsT=statD[k],
                        rhs=obs[:, k * S_ + blk * 512:
                                k * S_ + (blk + 1) * 512],
                        start=False, stop=(k == NDC - 1))
                nc.vector.tensor_reduce(out=ymax[:, 2 * b + blk:
                                            2 * b + blk + 1],
                                        in_=py[:], axis=X, op=MAX)
            nc.vector.tensor_reduce(out=lat128[:, b:b + 1],
                                    in_=ymax[:, 2 * b:2 * b + 2],
                                    axis=X, op=MAX)
            del st[b]

        # MLP head split into per-batch pipelined micro-stages so no PE op
        # ever waits on a fresh cross-engine chain.
        def mlp_a1(b):
            bc = slice(b, b + 1)
            nc.scalar.copy(out=lat128b[:, bc], in_=lat128[:, bc])
            psw = pWE.tile([128, 512], F32, tag="wE0", name="psw")
            nc.tensor.matmul(out=psw[:, 0:1], lhsT=permP,
                             rhs=lat128b[:, bc], start=True, stop=True)
            nc.vector.tensor_tensor(out=latf[:, bc], in0=lat128[0:64, bc],
                                    in1=psw[0:64, 0:1], op=MAX)

        def mlp_a2(b):
            bc = slice(b, b + 1)
            nc.scalar.copy(out=latfb[:, bc], in_=latf[:, bc])
            plw = pWE.tile([128, 512], F32, tag="wE1", name="plw")
            nc.tensor.matmul(out=plw[0:64, 0:1], lhsT=w1lat[0:64, :],
                             rhs=latfb[:, bc], start=True, stop=True)
            nc.scalar.activation(out=latW[:, bc], in_=plw[0:64, 0:1],
                                 func=mybir.ActivationFunctionType.Identity,
                                 bias=b1[:], scale=1.0)
            nc.scalar.activation(out=x1[:, b * A_:(b + 1) * A_],
                                 in_=xa[:, b * A_:(b + 1) * A_],
                                 func=TANH, bias=latW[:, bc], scale=1.0)

        def mlp_b(h):
            hl = slice(h * 512, (h + 1) * 512)
            px2 = pWE.tile([128, 512], F32, tag="wE0", name="px2")
            nc.tensor.matmul(out=px2[0:32, :], lhsT=w2[:], rhs=x1[:, hl],
                             start=True, stop=True)
            nc.scalar.activation(out=x2[:, hl], in_=px2[0:32, :],
                                 func=TANH, bias=b2[:], scale=1.0)
            px3 = pWE.tile([128, 512], F32, tag="wE1", name="px3")
            nc.tensor.matmul(out=px3[0:1, :], lhsT=w3[:], rhs=x2[:, hl],
                             start=True, stop=True)
            nc.scalar.activation(out=x3[:, hl], in_=px3[0:1, :],
                                 func=TANH, bias=b3[:], scale=1.0)

        for i in range(NB + 2):
            # MLP micro-stages first: all inputs are >= 1 iteration old, so
            # their short cross-engine chains finish early and never stall
            # the PE at the iteration tail.
            if 3 <= i <= 9:
                mlp_a1(i - 3)
            if 4 <= i <= 9:
                mlp_a2(i - 4)
            if i == 8:
                mlp_b(0)
            if i < NB - 2:
                load(i + 2)
            if i < NB:
                stage_statA_rot(i)
            if 1 <= i <= NB:
                stage_mix_scan(i - 1)
                stage_p(i - 1)
            if i == 3:
                emit_xa()
            if i >= 2:
                stage_y_red(i - 2)
        mlp_a1(7)
        mlp_a2(6)
        mlp_a2(7)
        mlp_b(1)

        nc.sync.dma_start(out=out_d, in_=x3[:])

    nc.compile()
    return nc


_NC_CACHE = {}


def _get_nc():
    if "nc" not in _NC_CACHE:
        _NC_CACHE["nc"] = _build_nc()
    return _NC_CACHE["nc"]


# de-interleave order: col c<1024 -> s=2c+1 (odd), c>=1024 -> s=2(c-1024)
_DEINT = np.concatenate([np.arange(1, S_, 2), np.arange(0, S_, 2)])


def _host_tables(nu_log, theta_log, gamma_log, B_re, B_im, C_re, C_im, D,
                 W1, b1, W2, b2, W3, b3):
    f64 = np.float64
    bf = ml_dtypes.bfloat16
    rho_h = np.exp(-np.exp(nu_log.astype(f64)))          # [H]
    theta_h = np.exp(theta_log.astype(f64))              # [H]
    gamma_h = np.exp(gamma_log.astype(f64))              # [H]
    rho128 = np.concatenate([rho_h, rho_h])              # [128]

    # table position map for uAc/t1 columns [W_=2050]:
    # col 0 -> pad(0); col j in 1..1024 -> s=2j-1; col 1025 -> pad(0);
    # col j in 1026..2049 -> s=2j-2052
    s_of = np.zeros(W_, dtype=f64)
    s_of[1:1025] = 2 * np.arange(1, 1025, dtype=f64) - 1
    s_of[1026:] = 2 * np.arange(1026, W_, dtype=f64) - 2052
    phase = (theta_h[:, None] * s_of[None, :]) % (2 * np.pi)   # [H, W_]
    cos_t = np.cos(phase)
    sin_t = np.sin(phase)
    # rho fold on decimation in0 cols (0..1023); zero the two pad cols
    cosDI = np.concatenate([cos_t, cos_t], axis=0)       # [128, W_]
    sinDI = np.concatenate([-sin_t, sin_t], axis=0)
    cosDI[:, 0:1024] *= rho128[:, None]
    sinDI[:, 0:1024] *= rho128[:, None]
    cosDI[:, 0] = 0.0
    cosDI[:, 1025] = 0.0
    sinDI[:, 0] = 0.0
    sinDI[:, 1025] = 0.0

    tabs = np.concatenate([cosDI, sinDI], axis=1).astype(bf)

    rho2f = np.broadcast_to((rho128 ** 2).astype(np.float32)[:, None],
                            (128, 512)).copy()

    Bg_re = (B_re.astype(f64) * gamma_h[:, None])        # [H, D_IN]
    Bg_im = (B_im.astype(f64) * gamma_h[:, None])
    statA = np.concatenate([Bg_re.T, Bg_im.T], axis=1)   # [D_IN, 128]
    statA = statA.reshape(NDC, 128, 128)
    permP = np.zeros((128, 128), dtype=f64)
    for m in range(128):
        permP[m ^ 64, m] = 1
    ident = np.eye(128, dtype=f64)
    statD = D.T.reshape(NDC, 128, D_OUT).astype(f64)

    cm1 = np.concatenate([C_re.T, -C_im.T], axis=0)
    # p2 uses the sinDI even half whose top 64 rows carry -sin, so the top
    # half of cm2/cm2l flips sign vs the plain [-C_im; -C_re] form.
    cm2 = np.concatenate([C_im.T, -C_re.T], axis=0)
    lam_re = rho_h * np.cos(theta_h)
    lam_im = rho_h * np.sin(theta_h)
    Cp_re = C_re.astype(f64) * lam_re[None, :] - C_im.astype(f64) * lam_im[None, :]
    Cp_im = C_re.astype(f64) * lam_im[None, :] + C_im.astype(f64) * lam_re[None, :]
    cm1l = np.concatenate([Cp_re.T, -Cp_im.T], axis=0)
    cm2l = np.concatenate([Cp_im.T, -Cp_re.T], axis=0)

    w1lat = np.zeros((128, 64), dtype=f64)
    w1lat[:H_] = W1[:, :H_].T
    w1act = W1[:, H_:].T.reshape(NDC, 128, D_MLP).astype(f64)

    stat = np.concatenate(
        [np.concatenate([statA[k] for k in range(NDC)], axis=1),
         permP, ident,
         np.concatenate([statD[k] for k in range(NDC)], axis=1),
         cm1, cm2, cm1l, cm2l, w1lat,
         np.concatenate([w1act[k] for k in range(NDC)], axis=1)],
        axis=1).astype(bf)
    assert stat.shape == (128, 1344), stat.shape

    return dict(
        tabs=tabs, rho2f=rho2f, stat=stat,
        w2=W2.T.astype(bf), w3=W3.T.astype(bf),
        b1=b1.reshape(64, 1).astype(np.float32),
        b2=b2.reshape(32, 1).astype(np.float32),
        b3=b3.reshape(1, 1).astype(np.float32),
    )


def kernel(observations, actions, nu_log, theta_log, gamma_log,
           B_re, B_im, C_re, C_im, D, W1, b1, W2, b2, W3, b3,
           _trace=False, _tmpdir=None):
    obs_bf = np.asarray(observations, dtype=np.float32).astype(
        ml_dtypes.bfloat16)
    act_bf = np.asarray(actions, dtype=np.float32).astype(ml_dtypes.bfloat16)
    # host-side transpose + de-interleave: obsT [B, NDC, 128, S_]
    obsT_all = np.ascontiguousarray(
        obs_bf.transpose(0, 2, 1)[:, :, _DEINT]).reshape(B_, NDC, 128, S_)
    tables = _host_tables(np.asarray(nu_log), np.asarray(theta_log),
                          np.asarray(gamma_log), np.asarray(B_re),
                          np.asarray(B_im), np.asarray(C_re),
                          np.asarray(C_im), np.asarray(D),
                          np.asarray(W1), np.asarray(b1), np.asarray(W2),
                          np.asarray(b2), np.asarray(W3), np.asarray(b3))
    in_maps = []
    for c in range(NCORES):
        m = dict(tables)
        m["obsT"] = np.ascontiguousarray(obsT_all[c * NB:(c + 1) * NB])
        act_c = act_bf[c * NB:(c + 1) * NB].reshape(NAB, D_IN)
        m["actT"] = np.ascontiguousarray(act_c.T).reshape(NDC, 128, NAB)
        in_maps.append(m)

    nc = _get_nc()
    res = run_bass_kernel_spmd(nc, in_maps, core_ids=list(range(NCORES)),
                               trace=_trace, tmpdir=_tmpdir)
    outs = []
    for c in range(NCORES):
        outs.append(np.asarray(res.results[c]["out"]).reshape(NB, A_, 1))
    full = np.concatenate(outs, axis=0).astype(np.float32)
    if _trace:
        return full, res
    return full
